# revision 29
# baseline (speedup 1.0000x reference)
"""Trainium2 Bass kernel for a 2-layer GCN (PyG GCNConv + dense layer).

Computation (matches the jax reference):
    deg[n]  = 1 + sum of incoming edge weights        (self loop weight 1)
    dinv    = deg ** -0.5
    norm_e  = dinv[src] * ew * dinv[dst]              (per edge, incl. self)
    agg[n]  = sum_e norm_e * x[src_e]                 (propagate FIRST: A(xW) == (Ax)W)
    h       = relu(agg @ W1 + b1)
    out     = relu(h @ W2 + b2)

Distribution: nodes (as scatter destinations) are partitioned across the 8
cores; each core gathers the bf16 x-rows for its incoming edges straight from
a replicated x table in HBM (one multi-packet dma_gather per 4-tile batch and
table half - the SWDGE fixed overhead of ~1us dominates per-instruction cost,
so gathers are batched as large as SBUF allows).  Each 128-edge chunk's
selection matrix S (holding the edge norms) is generated in bulk, one pair of
DVE tensor_tensor ops per batch using broadcast access patterns, and the PE
accumulates  msg^T @ S  into PSUM, giving the aggregation feature-major.
W1/W2 matmuls then run feature-major (f32r, 1 cyc/col at moving dim 512) with
nodes on the moving dimension.  The output stays feature-major in HBM; the
final transpose to node-major rows happens on the host together with the
tile-packing un-permutation.

Known HW constraints baked in: int16 gather indices cap table views at 32768
rows, so the table is stored even-nodes-first and edges split into A/B halves
by src parity; fp32 matmuls cost 4 cyc/col (hence bf16 scatter + f32r dense);
PSUM banks are 2KB/partition (each [128,512] f32 accumulator is one bank).

Host-side work is limited to graph preprocessing: self-loop append, degree /
norm computation (O(E) scalar ops), edge bucketing by destination tile, and
the final row un-permutation + transpose of the feature-major outputs.
"""

import os
import sys

import numpy as np

sys.path.insert(0, "/opt/trn_rl_repo")

P = 128
N_CORES = 8
HALF = 32768          # int16 index limit per gather table view
G_TILES = 4           # node tiles per batch (double buffered)

D_IN = 128
D_HID = 512
D_OUT = 128


def _greedy_tiles(cnt_a, cnt_b, n_tiles):
    """Assign local nodes to n_tiles bins of <=P nodes, jointly balancing the
    per-tile A-half and B-half incoming-edge counts (each half's max drives a
    padded chunk count for every tile on every core).
    Returns tile_of[node], pos_in_tile[node]."""
    n = len(cnt_a)
    tot = cnt_a + cnt_b
    order = np.argsort(-tot, kind="stable")
    tile_of = np.empty(n, np.int32)
    pos_in_tile = np.empty(n, np.int32)
    counts = np.zeros(n_tiles, np.int32)
    loadA = np.zeros(n_tiles, np.float64)
    loadB = np.zeros(n_tiles, np.float64)
    tgtA = max(1.0, cnt_a.sum() / n_tiles)
    tgtB = max(1.0, cnt_b.sum() / n_tiles)
    big = np.float64(1e18)
    for node in order:
        score = np.maximum((loadA + cnt_a[node]) / tgtA,
                           (loadB + cnt_b[node]) / tgtB)
        score = np.where(counts < P, score, big)
        t = int(np.argmin(score))
        tile_of[node] = t
        pos_in_tile[node] = counts[t]
        counts[t] += 1
        loadA[t] += cnt_a[node]
        loadB[t] += cnt_b[node]
    return tile_of, pos_in_tile


def _preprocess(x, edge_index, edge_weight):
    """Full-graph preprocessing; returns per-core packed arrays + layout."""
    N = x.shape[0]
    n_per = N // N_CORES
    assert n_per * N_CORES == N

    src = np.asarray(edge_index[0], np.int64)
    dst = np.asarray(edge_index[1], np.int64)
    ew = np.asarray(edge_weight, np.float32)
    ids = np.arange(N, dtype=np.int64)
    src_f = np.concatenate([src, ids])
    dst_f = np.concatenate([dst, ids])
    ew_f = np.concatenate([ew, np.ones(N, np.float32)])

    deg = np.bincount(dst_f, weights=ew_f.astype(np.float64), minlength=N)
    deg = deg.astype(np.float32)
    dinv = np.where(deg > 0, 1.0 / np.sqrt(deg), 0.0).astype(np.float32)
    norm = (ew_f * dinv[src_f] * dinv[dst_f]).astype(np.float32)

    n_tiles = -(-n_per // P)              # real tiles per core
    n_batches = -(-n_tiles // G_TILES)
    tiles_tot = n_batches * G_TILES       # padded tile count (ghost tiles)

    # Table views are int16-indexed (<=32768 rows each).  For N > HALF the
    # gather table is stored with even nodes first, odd nodes second, so BOTH
    # views are N/2 rows and every core's edge mix is ~50/50 across views.
    interleave = N > HALF
    rows_a = (N + 1) // 2 if interleave else N
    rows_b = N // 2 if interleave else 0

    cores = []
    for c in range(N_CORES):
        lo, hi = c * n_per, (c + 1) * n_per
        m = (dst_f >= lo) & (dst_f < hi)
        es = src_f[m]
        ed = (dst_f[m] - lo).astype(np.int64)
        en = norm[m]
        if interleave:
            e_half = (es % 2).astype(np.int64)      # odd src -> B view
            e_idx = (es // 2).astype(np.int64)
        else:
            e_half = np.zeros(len(es), np.int64)
            e_idx = es
        cnt_a = np.bincount(ed[e_half == 0], minlength=n_per)
        cnt_b = np.bincount(ed[e_half == 1], minlength=n_per)
        tile_of, pos_in_tile = _greedy_tiles(cnt_a, cnt_b, n_tiles)

        te = tile_of[ed]
        order = np.lexsort((e_idx, e_half, te))
        eidx = e_idx[order]
        ed, en, te, he = ed[order], en[order], te[order], e_half[order]

        seg = te * 2 + he                      # sorted ascending now
        seg_starts = np.searchsorted(seg, np.arange(tiles_tot * 2), side="left")
        rank = np.arange(len(eidx)) - seg_starts[seg]
        lenA = np.bincount(te[he == 0], minlength=tiles_tot)
        lenB = np.bincount(te[he == 1], minlength=tiles_tot)

        cores.append(dict(eidx=eidx, en=en, ed=ed, te=te, he=he, rank=rank,
                          lenA=lenA, lenB=lenB, tile_of=tile_of,
                          pos_in_tile=pos_in_tile, lo=lo))

    K_A = max(1, int(max(-(-core["lenA"].max() // P) for core in cores)))
    if interleave:
        K_B = max(1, int(max(-(-core["lenB"].max() // P) for core in cores)))
    else:
        K_B = 0
    K = K_A + K_B
    n_slots = tiles_tot * K

    per_core = []
    for core in cores:
        gidx = np.full(n_slots * P, -1, np.int16)       # -1 = skipped pad
        mnorm = np.zeros(n_slots * P, np.float32)
        mdst = np.zeros(n_slots * P, np.int32)

        te, he, rank = core["te"], core["he"], core["rank"]
        g = te // G_TILES
        tb = te % G_TILES
        jc = rank // P
        pp = rank % P
        bK = G_TILES * K
        slot = np.where(
            he == 0,
            g * bK + tb * K_A + jc,
            g * bK + G_TILES * K_A + tb * K_B + jc,
        )
        lin = slot * P + pp
        gidx[lin] = core["eidx"].astype(np.int16)
        mnorm[lin] = core["en"]
        mdst[lin] = core["pos_in_tile"][core["ed"]]

        # gather windows: one per (batch, half, tile), pads trailing.
        # cnt = number of real (non-negative) idx in each window (min 1:
        # an all-pad window keeps one idx=0 descriptor so the count is
        # never zero).
        n_win = n_batches * 2 * G_TILES
        cnt = np.zeros(n_win, np.int32)
        w = 0
        for gg in range(n_batches):
            for hs, k_h in ((0, K_A), (G_TILES * K_A, K_B)):
                for t in range(G_TILES):
                    s0 = gg * bK + hs + t * k_h
                    seg = gidx[s0 * P:(s0 + k_h) * P]
                    c = int((seg >= 0).sum())
                    assert (seg[:c] >= 0).all()         # pads trail
                    if c == 0:
                        seg[0] = 0
                        c = 1
                    cnt[w] = c
                    w += 1

        # index list wrapped into 16 partitions, replicated to 128
        g16 = gidx.reshape(-1, 16).T.copy()             # [16, n_slots*8]
        g128 = np.tile(g16, (8, 1))                     # [128, n_slots*8]

        # permutation: tile-slot row -> global node id (-1 for ghosts)
        perm = np.full(tiles_tot * P, -1, np.int64)
        node_rows = core["tile_of"].astype(np.int64) * P + core["pos_in_tile"]
        perm[node_rows] = np.arange(len(core["tile_of"])) + core["lo"]

        per_core.append(dict(
            gidx=g128,
            mnorm=mnorm.reshape(n_slots, P).T.copy(),   # [128, n_slots]
            mdst=mdst.astype(np.float32).reshape(n_slots, P).T.copy(),
            cnt=cnt,
            perm=perm,
        ))

    layout = dict(K_A=K_A, K_B=K_B, K=K, n_slots=n_slots,
                  n_batches=n_batches, tiles_tot=tiles_tot, n_tiles_real=n_tiles,
                  n_rows_A=rows_a, n_rows_B=rows_b)
    return per_core, layout


def _build_program(layout):
    from concourse import bacc, mybir, tile

    f32 = mybir.dt.float32
    bf16 = mybir.dt.bfloat16
    i16 = mybir.dt.int16
    K_A, K_B, K = layout["K_A"], layout["K_B"], layout["K"]
    n_batches = layout["n_batches"]
    n_slots = layout["n_slots"]
    tiles_tot = layout["tiles_tot"]
    N = layout["n_rows_A"] + layout["n_rows_B"]
    bK = G_TILES * K                  # slots per batch
    idx_cols = n_slots * P // 16

    i32 = mybir.dt.int32
    # cdata packs the f32 constants: w1(512) | w2r(4*128) | b1c(4) | b2c(1)
    O_W1, O_W2 = 0, 512
    O_B1, O_B2 = 1024, 1028
    C_COLS = 1029
    # cbf packs the bf16 S-gen constants: iota_rep (col-major, [c,k] -> c)
    # | mdst | mnorm.  All unit-step in the (col outer, slot inner) iteration
    # of the transposed S so the DVE 2x packed mode engages.
    O_IREP = 0
    O_MDST = bK * P
    O_MNORM = O_MDST + n_slots
    CB_COLS = O_MNORM + n_slots
    n_win = n_batches * 2 * G_TILES

    nc = bacc.Bacc("TRN2", num_swdge_queues=4,
                   dynamic_dma_scratch_size=65536)
    xtab = nc.declare_dram_parameter("xtab", [N, D_IN], bf16, isOutput=False)
    cdata_d = nc.declare_dram_parameter("cdata", [P, C_COLS], f32, isOutput=False)
    cbf_d = nc.declare_dram_parameter("cbf", [P, CB_COLS], bf16, isOutput=False)
    cnt_d = nc.declare_dram_parameter("cnt", [P, n_win], i32, isOutput=False)
    gidx_d = nc.declare_dram_parameter("gidx", [P, idx_cols], i16, isOutput=False)
    # feature-major output: column t*P+p is node-slot p of tile t
    out_d = nc.declare_dram_parameter("out", [P, tiles_tot * P], f32, isOutput=True)

    f32r = mybir.dt.float32r
    relu = mybir.ActivationFunctionType.Relu
    is_eq = mybir.AluOpType.is_equal
    mult = mybir.AluOpType.mult

    with tile.TileContext(nc) as tc:
        with (
            tc.tile_pool(name="const", bufs=1) as const,
            tc.tile_pool(name="gbuf", bufs=2) as gbuf,
            tc.tile_pool(name="spool", bufs=2) as spool,
            tc.tile_pool(name="aggp", bufs=2) as aggp,
            tc.tile_pool(name="hp", bufs=2) as hp,
            tc.tile_pool(name="outp", bufs=3) as outp,
            tc.tile_pool(name="psa", bufs=2, space="PSUM") as psa,
            tc.tile_pool(name="psh", bufs=2, space="PSUM") as psh,
            tc.tile_pool(name="pso", bufs=2, space="PSUM") as pso,
        ):
            # ---- constants: two packed DMAs + the int16 index stream ----
            cdata_s = const.tile([P, C_COLS], f32)
            nc.sync.dma_start(out=cdata_s[:], in_=cdata_d[:])
            cbf_s = const.tile([P, CB_COLS], bf16)
            nc.sync.dma_start(out=cbf_s[:], in_=cbf_d[:])
            gidx_s = const.tile([P, idx_cols], i16)
            nc.sync.dma_start(out=gidx_s[:], in_=gidx_d[:])

            # one-time f32 -> f32r rounding of the matmul weights (the BIR
            # verifier requires f32r matmul operands to come from a rounding
            # producer)
            wconv = const.tile([P, 1024], f32r)
            nc.vector.tensor_copy(out=wconv[:], in_=cdata_s[:, O_W1:O_W1 + 1024])

            def w1_sl(cc):
                return wconv[:, O_W1 + cc * P:O_W1 + (cc + 1) * P]

            def w2_sl(cc):
                return wconv[:, O_W2 + cc * P:O_W2 + (cc + 1) * P]

            def b1_sl(cc):
                return cdata_s[:, O_B1 + cc:O_B1 + cc + 1]

            b2_sl = cdata_s[:, O_B2:O_B2 + 1]
            irep3 = cbf_s[:, O_IREP:O_IREP + bK * P] \
                .rearrange("p (c k) -> p c k", k=bK)
            mdst_s = cbf_s[:, O_MDST:O_MDST + n_slots]
            mnorm_s = cbf_s[:, O_MNORM:O_MNORM + n_slots]
            cnt_s = const.tile([P, n_win], i32)
            nc.sync.dma_start(out=cnt_s[:], in_=cnt_d[:])

            n_tiles_real = layout["n_tiles_real"]
            gq = [0]
            creg = nc.gpsimd.alloc_register("gather_cnt")

            def emit_gathers(g, gb):
                # one gather per (half, tile) window; pads trail in each
                # window as -1 so num_idxs_reg (per-core count) skips them.
                n_rt = max(0, min(G_TILES, n_tiles_real - g * G_TILES))
                col0 = g * bK * 8
                w0 = g * 2 * G_TILES
                for hi, (k_h, hs) in enumerate(((K_A, 0),
                                                (K_B, G_TILES * K_A))):
                    for t in range(n_rt):
                        s0 = hs + t * k_h
                        ni = k_h * P
                        w = w0 + hi * G_TILES + t
                        nc.gpsimd.reg_load(creg, cnt_s[0:1, w:w + 1])
                        nc.gpsimd.dma_gather(
                            out_ap=gb[:, s0:s0 + k_h, :],
                            in_ap=(xtab[0:layout["n_rows_A"], :] if hi == 0
                                   else xtab[layout["n_rows_A"]:N, :]),
                            idxs_ap=gidx_s[:, col0 + s0 * 8:
                                           col0 + (s0 + k_h) * 8],
                            num_idxs=ni, num_idxs_reg=creg,
                            elem_size=D_IN, queue_num=gq[0] % 4,
                            single_packet=True,
                        )
                        gq[0] += 1

            def emit_sgen(g):
                # S for the whole batch, stored transposed [P, col, slot]:
                # all operands unit-step in the (col outer, slot inner)
                # iteration -> DVE 2x packed mode (~4.7us per op at bK=56)
                St = spool.tile([P, P, bK], bf16, tag="St")
                mdst_bc = mdst_s[:, g * bK:(g + 1) * bK] \
                    .unsqueeze(1).to_broadcast([P, P, bK])
                mnorm_bc = mnorm_s[:, g * bK:(g + 1) * bK] \
                    .unsqueeze(1).to_broadcast([P, P, bK])
                nc.vector.tensor_tensor(out=St[:], in0=irep3, in1=mdst_bc,
                                        op=is_eq)
                nc.vector.tensor_tensor(out=St[:], in0=St[:], in1=mnorm_bc,
                                        op=mult)
                return St

            def emit_scatter(g, gb, St):
                n_rt = max(0, min(G_TILES, n_tiles_real - g * G_TILES))
                pagg = psa.tile([P, G_TILES * P], f32, space="PSUM")
                if n_rt < G_TILES:
                    # ghost-tile columns get no matmuls; init them so the
                    # group-wide eviction reads defined data
                    nc.vector.memset(pagg[:, n_rt * P:], 0)
                for tb in range(n_rt):
                    chunks = (
                        [tb * K_A + j for j in range(K_A)]
                        + [G_TILES * K_A + tb * K_B + j for j in range(K_B)]
                    )
                    for j, sl in enumerate(chunks):
                        nc.tensor.matmul(
                            out=pagg[:, tb * P:(tb + 1) * P],
                            lhsT=gb[:, sl, :],
                            rhs=St[:, :, sl],
                            start=(j == 0),
                            stop=(j == len(chunks) - 1),
                        )
                return pagg

            def emit_dense(g, pagg):
                aggT = aggp.tile([P, G_TILES * P], f32r)
                # PSUM->SBUF eviction on the scalar engine: keeps the DVE
                # free for S generation (its 2x tensor_tensor pair is the
                # densest per-batch DVE work)
                nc.scalar.copy(out=aggT[:], in_=pagg[:])
                # layer 1: hT[c] = relu(W1c^T @ aggT + b1c)
                hT = hp.tile([P, 4, G_TILES * P], f32r)
                for cc in range(4):
                    ph = psh.tile([P, G_TILES * P], f32, space="PSUM")
                    nc.tensor.matmul(out=ph[:], lhsT=w1_sl(cc), rhs=aggT[:],
                                     start=True, stop=True)
                    nc.scalar.activation(out=hT[:, cc, :], in_=ph[:],
                                         func=relu, bias=b1_sl(cc), scale=1.0)
                # layer 2: outT = relu(sum_c W2c^T @ hT[c] + b2)
                po = pso.tile([P, G_TILES * P], f32, space="PSUM")
                for cc in range(4):
                    nc.tensor.matmul(out=po[:], lhsT=w2_sl(cc),
                                     rhs=hT[:, cc, :],
                                     start=(cc == 0), stop=(cc == 3))
                outT = outp.tile([P, G_TILES * P], f32, tag="outT")
                nc.scalar.activation(out=outT[:], in_=po[:], func=relu,
                                     bias=b2_sl, scale=1.0)
                # write feature-major; host transposes + un-permutes
                nc.sync.dma_start(
                    out=out_d[:, g * G_TILES * P:(g + 1) * G_TILES * P],
                    in_=outT[:],
                )

            # software pipeline: dense for batch g-1 sits between batch g's
            # scatter matmuls in PE program order, so the PE never idles
            # long enough for the HAM clock gate to re-throttle it.
            # two explicit gather buffers, alternated manually: slots skipped
            # by the per-core descriptor counts keep stale-but-finite data
            # from two batches ago and S=0 masks them; the one-time memsets
            # make the first touch finite too
            gb_a = gbuf.tile([P, bK, D_IN], bf16, tag="gb_a")
            gb_b = gbuf.tile([P, bK, D_IN], bf16, tag="gb_b")
            gbs = [gb_a, gb_b]
            for t in gbs:
                nc.vector.memset(t[:], 0)
            emit_gathers(0, gbs[0])
            s_live = {0: emit_sgen(0)}    # S prefetched one batch ahead
            prev = None                   # (g, pagg) awaiting dense
            for g in range(n_batches):
                if g + 1 < n_batches:
                    emit_gathers(g + 1, gbs[(g + 1) % 2])
                if prev is not None:
                    emit_dense(*prev)
                if g + 1 < n_batches:
                    s_live[g + 1] = emit_sgen(g + 1)
                pagg = emit_scatter(g, gbs[g % 2], s_live.pop(g))
                prev = (g, pagg)
            emit_dense(*prev)

    nc.compile()
    return nc


def _install_ntff_hook():
    """The agent image's antenv lacks axon_hooks; fabricate it so trace=True
    can drive NTFF profiling through libaxon_pjrt.so's C ABI."""
    import contextlib
    import ctypes
    import types

    if "antenv.axon_hooks" in sys.modules:
        return
    so_path = "/opt/axon/libaxon_pjrt.so"
    if not os.path.exists(so_path):
        return
    lib = ctypes.CDLL(so_path)
    if not hasattr(lib, "axon_start_nrt_profile"):
        return
    lib.axon_start_nrt_profile.argtypes = [
        ctypes.POINTER(ctypes.c_int64), ctypes.c_size_t]
    lib.axon_start_nrt_profile.restype = ctypes.c_int64
    lib.axon_stop_nrt_profile.argtypes = [ctypes.c_char_p]
    lib.axon_stop_nrt_profile.restype = ctypes.c_int64

    @contextlib.contextmanager
    def _hook(output_dir, device_ids):
        import jax
        jax.devices()
        if device_ids:
            ids = (ctypes.c_int64 * len(device_ids))(*device_ids)
            rc = lib.axon_start_nrt_profile(ids, len(device_ids))
        else:
            rc = lib.axon_start_nrt_profile(None, 0)
        if rc != 0:
            raise RuntimeError(f"axon_start_nrt_profile rc={rc}")
        try:
            yield
        finally:
            n = lib.axon_stop_nrt_profile(str(output_dir).encode())
            print(f"ntff profile: {n} file(s) written to {output_dir}",
                  file=sys.stderr)

    import antenv  # noqa: F401
    mod = types.ModuleType("antenv.axon_hooks")
    mod._hook = _hook
    mod.set_axon_ntff_profile_hook = lambda h: setattr(mod, "_hook", h)
    mod.get_axon_ntff_profile_hook = lambda: mod._hook
    sys.modules["antenv.axon_hooks"] = mod


def _assemble_inputs(x, W1, b1, W2, b2, per_core, layout):
    import ml_dtypes

    bK = G_TILES * layout["K"]
    # iota_rep[p, c*bK + k] = c  (value c repeated along the slot axis)
    iota_rep = np.tile(np.repeat(np.arange(P, dtype=np.float32), bK), (P, 1))
    w2r = W2.reshape(4, P, D_OUT).transpose(1, 0, 2).reshape(P, 4 * D_OUT)
    b1c = b1.reshape(4, P).T
    b2c = b2.reshape(P, 1)

    N = x.shape[0]
    if N > HALF:
        # even nodes first, odd nodes second (matches _preprocess views)
        xt = np.empty_like(x)
        xt[:(N + 1) // 2] = x[0::2]
        xt[(N + 1) // 2:] = x[1::2]
    else:
        xt = x
    xtab_arr = np.ascontiguousarray(xt.astype(ml_dtypes.bfloat16))

    cdata = np.concatenate([W1, w2r, b1c, b2c], axis=1).astype(np.float32)
    cdata = np.ascontiguousarray(cdata)
    in_maps = []
    for pc in per_core:
        cbf = np.concatenate([iota_rep, pc["mdst"], pc["mnorm"]], axis=1)
        in_maps.append({
            "xtab": xtab_arr,
            "cdata": cdata,
            "cbf": np.ascontiguousarray(cbf.astype(ml_dtypes.bfloat16)),
            "cnt": np.ascontiguousarray(
                np.tile(pc["cnt"], (P, 1)).astype(np.int32)),
            "gidx": pc["gidx"],
        })
    return in_maps


def _run(nc, in_maps, trace=False):
    if trace:
        try:
            _install_ntff_hook()
        except Exception as e:  # degrade to untraced run
            print(f"ntff hook install failed: {e}", file=sys.stderr)
    from concourse.bass_utils import run_bass_kernel_spmd

    return run_bass_kernel_spmd(
        nc, in_maps, core_ids=list(range(N_CORES)), trace=trace,
    )


def kernel(x, edge_index, edge_weight, W1, b1, W2, b2, _want_trace=False):
    x = np.ascontiguousarray(np.asarray(x, np.float32))
    W1 = np.asarray(W1, np.float32)
    b1 = np.asarray(b1, np.float32)
    W2 = np.asarray(W2, np.float32)
    b2 = np.asarray(b2, np.float32)

    N = x.shape[0]
    per_core, layout = _preprocess(x, edge_index, edge_weight)
    nc = _build_program(layout)

    in_maps = _assemble_inputs(x, W1, b1, W2, b2, per_core, layout)
    res = _run(nc, in_maps, trace=_want_trace)

    out = np.empty((N, D_IN), np.float32)
    for c in range(N_CORES):
        cols = res.results[c]["out"]            # [128, tiles_tot*128] f-major
        perm = per_core[c]["perm"]
        valid = perm >= 0
        out[perm[valid]] = cols[:, valid].T

    kernel.last_results = res
    return out


# revision 33
# speedup vs baseline: 1.0092x; 1.0092x over previous
"""Trainium2 Bass kernel for a 2-layer GCN (PyG GCNConv + dense layer).

Computation (matches the jax reference):
    deg[n]  = 1 + sum of incoming edge weights        (self loop weight 1)
    dinv    = deg ** -0.5
    norm_e  = dinv[src] * ew * dinv[dst]              (per edge, incl. self)
    agg[n]  = sum_e norm_e * x[src_e]                 (propagate FIRST: A(xW) == (Ax)W)
    h       = relu(agg @ W1 + b1)
    out     = relu(h @ W2 + b2)

Distribution: nodes (as scatter destinations) are partitioned across the 8
cores; each core gathers the bf16 x-rows for its incoming edges straight from
a replicated x table in HBM.  Gathers are one single-packet dma_gather per
(batch, table-half, tile) window rotating the 4 SWDGE queues: a queue ring
drains at ~8ns/descriptor, so the 4-way rotation sets the gather floor, and
per-core num_idxs_reg counts skip the trailing -1 pad descriptors in each
window.  Each 128-edge chunk's selection matrix S (holding the edge norms) is
generated one batch ahead on the DVE as a TRANSPOSED [P, col, slot] tile via
two tensor_tensor ops whose operands are all unit-step in the (col outer,
slot inner) iteration (materialized iota_rep + mdst/mnorm broadcasts), which
engages the DVE 2x packed mode.  The PE accumulates  msg^T @ S  into PSUM
feature-major, with each batch's dense W1/W2 stage (f32r, moving dim 512)
software-pipelined between the next batch's scatter matmuls so the PE never
idles into a HAM re-throttle window.  PSUM eviction runs on the scalar
engine to keep the DVE free.  The output stays feature-major in HBM; the
final transpose to node-major rows happens on the host together with the
tile-packing un-permutation.

Known HW constraints baked in: int16 gather indices cap table views at 32768
rows, so the table is stored even-nodes-first and edges split into A/B halves
by src parity; fp32 matmuls cost 4 cyc/col (hence bf16 scatter + f32r dense,
with producers rounding to f32r for the BIR verifier); PSUM banks are
2KB/partition; broadcast (step-0 last dim) APs disable the DVE packed modes;
skipped pad gather slots keep stale-but-finite data that S=0 masks (the two
gather buffers are zeroed once so the first touch is finite).

Host-side work is limited to graph preprocessing: self-loop append, degree /
norm computation (O(E) scalar ops), edge bucketing by destination tile, and
the final row un-permutation + transpose of the feature-major outputs.
"""

import os
import sys

import numpy as np

sys.path.insert(0, "/opt/trn_rl_repo")

P = 128
N_CORES = 8
HALF = 32768          # int16 index limit per gather table view
G_TILES = 4           # node tiles per batch (double buffered)

D_IN = 128
D_HID = 512
D_OUT = 128


def _greedy_tiles(cnt_a, cnt_b, n_tiles):
    """Assign local nodes to n_tiles bins of <=P nodes, jointly balancing the
    per-tile A-half and B-half incoming-edge counts (each half's max drives a
    padded chunk count for every tile on every core).
    Returns tile_of[node], pos_in_tile[node]."""
    n = len(cnt_a)
    tot = cnt_a + cnt_b
    order = np.argsort(-tot, kind="stable")
    tile_of = np.empty(n, np.int32)
    pos_in_tile = np.empty(n, np.int32)
    counts = np.zeros(n_tiles, np.int32)
    loadA = np.zeros(n_tiles, np.float64)
    loadB = np.zeros(n_tiles, np.float64)
    tgtA = max(1.0, cnt_a.sum() / n_tiles)
    tgtB = max(1.0, cnt_b.sum() / n_tiles)
    big = np.float64(1e18)
    for node in order:
        score = np.maximum((loadA + cnt_a[node]) / tgtA,
                           (loadB + cnt_b[node]) / tgtB)
        score = np.where(counts < P, score, big)
        t = int(np.argmin(score))
        tile_of[node] = t
        pos_in_tile[node] = counts[t]
        counts[t] += 1
        loadA[t] += cnt_a[node]
        loadB[t] += cnt_b[node]
    return tile_of, pos_in_tile


def _preprocess(x, edge_index, edge_weight):
    """Full-graph preprocessing; returns per-core packed arrays + layout."""
    N = x.shape[0]
    n_per = N // N_CORES
    assert n_per * N_CORES == N

    src = np.asarray(edge_index[0], np.int64)
    dst = np.asarray(edge_index[1], np.int64)
    ew = np.asarray(edge_weight, np.float32)
    ids = np.arange(N, dtype=np.int64)
    src_f = np.concatenate([src, ids])
    dst_f = np.concatenate([dst, ids])
    ew_f = np.concatenate([ew, np.ones(N, np.float32)])

    deg = np.bincount(dst_f, weights=ew_f.astype(np.float64), minlength=N)
    deg = deg.astype(np.float32)
    dinv = np.where(deg > 0, 1.0 / np.sqrt(deg), 0.0).astype(np.float32)
    norm = (ew_f * dinv[src_f] * dinv[dst_f]).astype(np.float32)

    n_tiles = -(-n_per // P)              # real tiles per core
    n_batches = -(-n_tiles // G_TILES)
    tiles_tot = n_batches * G_TILES       # padded tile count (ghost tiles)

    # Table views are int16-indexed (<=32768 rows each).  For N > HALF the
    # gather table is stored with even nodes first, odd nodes second, so BOTH
    # views are N/2 rows and every core's edge mix is ~50/50 across views.
    interleave = N > HALF
    rows_a = (N + 1) // 2 if interleave else N
    rows_b = N // 2 if interleave else 0

    cores = []
    for c in range(N_CORES):
        lo, hi = c * n_per, (c + 1) * n_per
        m = (dst_f >= lo) & (dst_f < hi)
        es = src_f[m]
        ed = (dst_f[m] - lo).astype(np.int64)
        en = norm[m]
        if interleave:
            e_half = (es % 2).astype(np.int64)      # odd src -> B view
            e_idx = (es // 2).astype(np.int64)
        else:
            e_half = np.zeros(len(es), np.int64)
            e_idx = es
        cnt_a = np.bincount(ed[e_half == 0], minlength=n_per)
        cnt_b = np.bincount(ed[e_half == 1], minlength=n_per)
        tile_of, pos_in_tile = _greedy_tiles(cnt_a, cnt_b, n_tiles)

        te = tile_of[ed]
        order = np.lexsort((e_idx, e_half, te))
        eidx = e_idx[order]
        ed, en, te, he = ed[order], en[order], te[order], e_half[order]

        seg = te * 2 + he                      # sorted ascending now
        seg_starts = np.searchsorted(seg, np.arange(tiles_tot * 2), side="left")
        rank = np.arange(len(eidx)) - seg_starts[seg]
        lenA = np.bincount(te[he == 0], minlength=tiles_tot)
        lenB = np.bincount(te[he == 1], minlength=tiles_tot)

        cores.append(dict(eidx=eidx, en=en, ed=ed, te=te, he=he, rank=rank,
                          lenA=lenA, lenB=lenB, tile_of=tile_of,
                          pos_in_tile=pos_in_tile, lo=lo))

    K_A = max(1, int(max(-(-core["lenA"].max() // P) for core in cores)))
    if interleave:
        K_B = max(1, int(max(-(-core["lenB"].max() // P) for core in cores)))
    else:
        K_B = 0
    K = K_A + K_B
    n_slots = tiles_tot * K

    per_core = []
    for core in cores:
        gidx = np.full(n_slots * P, -1, np.int16)       # -1 = skipped pad
        mnorm = np.zeros(n_slots * P, np.float32)
        mdst = np.zeros(n_slots * P, np.int32)

        te, he, rank = core["te"], core["he"], core["rank"]
        g = te // G_TILES
        tb = te % G_TILES
        jc = rank // P
        pp = rank % P
        bK = G_TILES * K
        slot = np.where(
            he == 0,
            g * bK + tb * K_A + jc,
            g * bK + G_TILES * K_A + tb * K_B + jc,
        )
        lin = slot * P + pp
        gidx[lin] = core["eidx"].astype(np.int16)
        mnorm[lin] = core["en"]
        mdst[lin] = core["pos_in_tile"][core["ed"]]

        # gather windows: one per (batch, half, tile), pads trailing.
        # cnt = number of real (non-negative) idx in each window (min 1:
        # an all-pad window keeps one idx=0 descriptor so the count is
        # never zero).
        n_win = n_batches * 2 * G_TILES
        cnt = np.zeros(n_win, np.int32)
        w = 0
        for gg in range(n_batches):
            for hs, k_h in ((0, K_A), (G_TILES * K_A, K_B)):
                for t in range(G_TILES):
                    s0 = gg * bK + hs + t * k_h
                    seg = gidx[s0 * P:(s0 + k_h) * P]
                    c = int((seg >= 0).sum())
                    assert (seg[:c] >= 0).all()         # pads trail
                    if c == 0:
                        seg[0] = 0
                        c = 1
                    cnt[w] = c
                    w += 1

        # index list wrapped into 16 partitions, replicated to 128
        g16 = gidx.reshape(-1, 16).T.copy()             # [16, n_slots*8]
        g128 = np.tile(g16, (8, 1))                     # [128, n_slots*8]

        # permutation: tile-slot row -> global node id (-1 for ghosts)
        perm = np.full(tiles_tot * P, -1, np.int64)
        node_rows = core["tile_of"].astype(np.int64) * P + core["pos_in_tile"]
        perm[node_rows] = np.arange(len(core["tile_of"])) + core["lo"]

        per_core.append(dict(
            gidx=g128,
            mnorm=mnorm.reshape(n_slots, P).T.copy(),   # [128, n_slots]
            mdst=mdst.astype(np.float32).reshape(n_slots, P).T.copy(),
            cnt=cnt,
            perm=perm,
        ))

    layout = dict(K_A=K_A, K_B=K_B, K=K, n_slots=n_slots,
                  n_batches=n_batches, tiles_tot=tiles_tot, n_tiles_real=n_tiles,
                  n_rows_A=rows_a, n_rows_B=rows_b)
    return per_core, layout


def _build_program(layout):
    from concourse import bacc, mybir, tile

    f32 = mybir.dt.float32
    bf16 = mybir.dt.bfloat16
    i16 = mybir.dt.int16
    K_A, K_B, K = layout["K_A"], layout["K_B"], layout["K"]
    n_batches = layout["n_batches"]
    n_slots = layout["n_slots"]
    tiles_tot = layout["tiles_tot"]
    N = layout["n_rows_A"] + layout["n_rows_B"]
    bK = G_TILES * K                  # slots per batch
    idx_cols = n_slots * P // 16

    i32 = mybir.dt.int32
    # cdata packs the f32 constants: w1(512) | w2r(4*128) | b1c(4) | b2c(1)
    O_W1, O_W2 = 0, 512
    O_B1, O_B2 = 1024, 1028
    C_COLS = 1029
    # cbf packs the bf16 S-gen constants: iota_rep (col-major, [c,k] -> c)
    # | mdst | mnorm.  All unit-step in the (col outer, slot inner) iteration
    # of the transposed S so the DVE 2x packed mode engages.
    O_IREP = 0
    O_MDST = bK * P
    O_MNORM = O_MDST + n_slots
    CB_COLS = O_MNORM + n_slots
    n_win = n_batches * 2 * G_TILES

    nc = bacc.Bacc("TRN2", num_swdge_queues=4,
                   dynamic_dma_scratch_size=65536)
    xtab = nc.declare_dram_parameter("xtab", [N, D_IN], bf16, isOutput=False)
    cdata_d = nc.declare_dram_parameter("cdata", [P, C_COLS], f32, isOutput=False)
    cbf_d = nc.declare_dram_parameter("cbf", [P, CB_COLS], bf16, isOutput=False)
    cnt_d = nc.declare_dram_parameter("cnt", [P, n_win], i32, isOutput=False)
    gidx_d = nc.declare_dram_parameter("gidx", [P, idx_cols], i16, isOutput=False)
    # feature-major output: column t*P+p is node-slot p of tile t
    out_d = nc.declare_dram_parameter("out", [P, tiles_tot * P], f32, isOutput=True)

    f32r = mybir.dt.float32r
    relu = mybir.ActivationFunctionType.Relu
    is_eq = mybir.AluOpType.is_equal
    mult = mybir.AluOpType.mult

    with tile.TileContext(nc) as tc:
        with (
            tc.tile_pool(name="const", bufs=1) as const,
            tc.tile_pool(name="gbuf", bufs=2) as gbuf,
            tc.tile_pool(name="spool", bufs=2) as spool,
            tc.tile_pool(name="aggp", bufs=2) as aggp,
            tc.tile_pool(name="hp", bufs=2) as hp,
            tc.tile_pool(name="outp", bufs=3) as outp,
            tc.tile_pool(name="psa", bufs=2, space="PSUM") as psa,
            tc.tile_pool(name="psh", bufs=2, space="PSUM") as psh,
            tc.tile_pool(name="pso", bufs=2, space="PSUM") as pso,
        ):
            # ---- constants: two packed DMAs + the int16 index stream ----
            cdata_s = const.tile([P, C_COLS], f32)
            nc.sync.dma_start(out=cdata_s[:], in_=cdata_d[:])
            cbf_s = const.tile([P, CB_COLS], bf16)
            nc.sync.dma_start(out=cbf_s[:], in_=cbf_d[:])
            gidx_s = const.tile([P, idx_cols], i16)
            # per-batch slices: batch 0's gathers start as soon as its own
            # index columns land instead of after the whole 1.4MB stream
            bcols = bK * 8
            for g0 in range(n_batches):
                nc.sync.dma_start(
                    out=gidx_s[:, g0 * bcols:(g0 + 1) * bcols],
                    in_=gidx_d[:, g0 * bcols:(g0 + 1) * bcols])

            # one-time f32 -> f32r rounding of the matmul weights (the BIR
            # verifier requires f32r matmul operands to come from a rounding
            # producer)
            wconv = const.tile([P, 1024], f32r)
            nc.vector.tensor_copy(out=wconv[:], in_=cdata_s[:, O_W1:O_W1 + 1024])

            def w1_sl(cc):
                return wconv[:, O_W1 + cc * P:O_W1 + (cc + 1) * P]

            def w2_sl(cc):
                return wconv[:, O_W2 + cc * P:O_W2 + (cc + 1) * P]

            def b1_sl(cc):
                return cdata_s[:, O_B1 + cc:O_B1 + cc + 1]

            b2_sl = cdata_s[:, O_B2:O_B2 + 1]
            irep3 = cbf_s[:, O_IREP:O_IREP + bK * P] \
                .rearrange("p (c k) -> p c k", k=bK)
            mdst_s = cbf_s[:, O_MDST:O_MDST + n_slots]
            mnorm_s = cbf_s[:, O_MNORM:O_MNORM + n_slots]
            cnt_s = const.tile([P, n_win], i32)
            nc.sync.dma_start(out=cnt_s[:], in_=cnt_d[:])

            n_tiles_real = layout["n_tiles_real"]
            gq = [0]
            creg = nc.gpsimd.alloc_register("gather_cnt")

            def emit_gathers(g, gb):
                # one gather per (half, tile) window; pads trail in each
                # window as -1 so num_idxs_reg (per-core count) skips them.
                n_rt = max(0, min(G_TILES, n_tiles_real - g * G_TILES))
                col0 = g * bK * 8
                w0 = g * 2 * G_TILES
                for hi, (k_h, hs) in enumerate(((K_A, 0),
                                                (K_B, G_TILES * K_A))):
                    for t in range(n_rt):
                        s0 = hs + t * k_h
                        ni = k_h * P
                        w = w0 + hi * G_TILES + t
                        nc.gpsimd.reg_load(creg, cnt_s[0:1, w:w + 1])
                        nc.gpsimd.dma_gather(
                            out_ap=gb[:, s0:s0 + k_h, :],
                            in_ap=(xtab[0:layout["n_rows_A"], :] if hi == 0
                                   else xtab[layout["n_rows_A"]:N, :]),
                            idxs_ap=gidx_s[:, col0 + s0 * 8:
                                           col0 + (s0 + k_h) * 8],
                            num_idxs=ni, num_idxs_reg=creg,
                            elem_size=D_IN, queue_num=gq[0] % 4,
                            single_packet=True,
                        )
                        gq[0] += 1

            def emit_sgen(g):
                # S for the whole batch, stored transposed [P, col, slot]:
                # all operands unit-step in the (col outer, slot inner)
                # iteration -> DVE 2x packed mode (~4.7us per op at bK=56)
                St = spool.tile([P, P, bK], bf16, tag="St")
                mdst_bc = mdst_s[:, g * bK:(g + 1) * bK] \
                    .unsqueeze(1).to_broadcast([P, P, bK])
                mnorm_bc = mnorm_s[:, g * bK:(g + 1) * bK] \
                    .unsqueeze(1).to_broadcast([P, P, bK])
                nc.vector.tensor_tensor(out=St[:], in0=irep3, in1=mdst_bc,
                                        op=is_eq)
                nc.vector.tensor_tensor(out=St[:], in0=St[:], in1=mnorm_bc,
                                        op=mult)
                return St

            def emit_scatter(g, gb, St):
                n_rt = max(0, min(G_TILES, n_tiles_real - g * G_TILES))
                pagg = psa.tile([P, G_TILES * P], f32, space="PSUM")
                if n_rt < G_TILES:
                    # ghost-tile columns get no matmuls; init them so the
                    # group-wide eviction reads defined data
                    nc.vector.memset(pagg[:, n_rt * P:], 0)
                for tb in range(n_rt):
                    chunks = (
                        [tb * K_A + j for j in range(K_A)]
                        + [G_TILES * K_A + tb * K_B + j for j in range(K_B)]
                    )
                    for j, sl in enumerate(chunks):
                        nc.tensor.matmul(
                            out=pagg[:, tb * P:(tb + 1) * P],
                            lhsT=gb[:, sl, :],
                            rhs=St[:, :, sl],
                            start=(j == 0),
                            stop=(j == len(chunks) - 1),
                        )
                return pagg

            def emit_dense(g, pagg):
                aggT = aggp.tile([P, G_TILES * P], f32r)
                # PSUM->SBUF eviction on the scalar engine: keeps the DVE
                # free for S generation (its 2x tensor_tensor pair is the
                # densest per-batch DVE work)
                nc.scalar.copy(out=aggT[:], in_=pagg[:])
                # layer 1: hT[c] = relu(W1c^T @ aggT + b1c)
                hT = hp.tile([P, 4, G_TILES * P], f32r)
                for cc in range(4):
                    ph = psh.tile([P, G_TILES * P], f32, space="PSUM")
                    nc.tensor.matmul(out=ph[:], lhsT=w1_sl(cc), rhs=aggT[:],
                                     start=True, stop=True)
                    nc.scalar.activation(out=hT[:, cc, :], in_=ph[:],
                                         func=relu, bias=b1_sl(cc), scale=1.0)
                # layer 2: outT = relu(sum_c W2c^T @ hT[c] + b2)
                po = pso.tile([P, G_TILES * P], f32, space="PSUM")
                for cc in range(4):
                    nc.tensor.matmul(out=po[:], lhsT=w2_sl(cc),
                                     rhs=hT[:, cc, :],
                                     start=(cc == 0), stop=(cc == 3))
                outT = outp.tile([P, G_TILES * P], f32, tag="outT")
                nc.scalar.activation(out=outT[:], in_=po[:], func=relu,
                                     bias=b2_sl, scale=1.0)
                # write feature-major; host transposes + un-permutes
                nc.sync.dma_start(
                    out=out_d[:, g * G_TILES * P:(g + 1) * G_TILES * P],
                    in_=outT[:],
                )

            # software pipeline: dense for batch g-1 sits between batch g's
            # scatter matmuls in PE program order, so the PE never idles
            # long enough for the HAM clock gate to re-throttle it.
            # two explicit gather buffers, alternated manually: slots skipped
            # by the per-core descriptor counts keep stale-but-finite data
            # from two batches ago and S=0 masks them; the one-time memsets
            # make the first touch finite too
            gb_a = gbuf.tile([P, bK, D_IN], bf16, tag="gb_a")
            gb_b = gbuf.tile([P, bK, D_IN], bf16, tag="gb_b")
            gbs = [gb_a, gb_b]
            for t in gbs:
                nc.vector.memset(t[:], 0)
            emit_gathers(0, gbs[0])
            s_live = {0: emit_sgen(0)}    # S prefetched one batch ahead
            prev = None                   # (g, pagg) awaiting dense
            for g in range(n_batches):
                if g + 1 < n_batches:
                    emit_gathers(g + 1, gbs[(g + 1) % 2])
                if prev is not None:
                    emit_dense(*prev)
                if g + 1 < n_batches:
                    s_live[g + 1] = emit_sgen(g + 1)
                pagg = emit_scatter(g, gbs[g % 2], s_live.pop(g))
                prev = (g, pagg)
            emit_dense(*prev)

    nc.compile()
    return nc


def _install_ntff_hook():
    """The agent image's antenv lacks axon_hooks; fabricate it so trace=True
    can drive NTFF profiling through libaxon_pjrt.so's C ABI."""
    import contextlib
    import ctypes
    import types

    if "antenv.axon_hooks" in sys.modules:
        return
    so_path = "/opt/axon/libaxon_pjrt.so"
    if not os.path.exists(so_path):
        return
    lib = ctypes.CDLL(so_path)
    if not hasattr(lib, "axon_start_nrt_profile"):
        return
    lib.axon_start_nrt_profile.argtypes = [
        ctypes.POINTER(ctypes.c_int64), ctypes.c_size_t]
    lib.axon_start_nrt_profile.restype = ctypes.c_int64
    lib.axon_stop_nrt_profile.argtypes = [ctypes.c_char_p]
    lib.axon_stop_nrt_profile.restype = ctypes.c_int64

    @contextlib.contextmanager
    def _hook(output_dir, device_ids):
        import jax
        jax.devices()
        if device_ids:
            ids = (ctypes.c_int64 * len(device_ids))(*device_ids)
            rc = lib.axon_start_nrt_profile(ids, len(device_ids))
        else:
            rc = lib.axon_start_nrt_profile(None, 0)
        if rc != 0:
            raise RuntimeError(f"axon_start_nrt_profile rc={rc}")
        try:
            yield
        finally:
            n = lib.axon_stop_nrt_profile(str(output_dir).encode())
            print(f"ntff profile: {n} file(s) written to {output_dir}",
                  file=sys.stderr)

    import antenv  # noqa: F401
    mod = types.ModuleType("antenv.axon_hooks")
    mod._hook = _hook
    mod.set_axon_ntff_profile_hook = lambda h: setattr(mod, "_hook", h)
    mod.get_axon_ntff_profile_hook = lambda: mod._hook
    sys.modules["antenv.axon_hooks"] = mod


def _assemble_inputs(x, W1, b1, W2, b2, per_core, layout):
    import ml_dtypes

    bK = G_TILES * layout["K"]
    # iota_rep[p, c*bK + k] = c  (value c repeated along the slot axis)
    iota_rep = np.tile(np.repeat(np.arange(P, dtype=np.float32), bK), (P, 1))
    w2r = W2.reshape(4, P, D_OUT).transpose(1, 0, 2).reshape(P, 4 * D_OUT)
    b1c = b1.reshape(4, P).T
    b2c = b2.reshape(P, 1)

    N = x.shape[0]
    if N > HALF:
        # even nodes first, odd nodes second (matches _preprocess views)
        xt = np.empty_like(x)
        xt[:(N + 1) // 2] = x[0::2]
        xt[(N + 1) // 2:] = x[1::2]
    else:
        xt = x
    xtab_arr = np.ascontiguousarray(xt.astype(ml_dtypes.bfloat16))

    cdata = np.concatenate([W1, w2r, b1c, b2c], axis=1).astype(np.float32)
    cdata = np.ascontiguousarray(cdata)
    in_maps = []
    for pc in per_core:
        cbf = np.concatenate([iota_rep, pc["mdst"], pc["mnorm"]], axis=1)
        in_maps.append({
            "xtab": xtab_arr,
            "cdata": cdata,
            "cbf": np.ascontiguousarray(cbf.astype(ml_dtypes.bfloat16)),
            "cnt": np.ascontiguousarray(
                np.tile(pc["cnt"], (P, 1)).astype(np.int32)),
            "gidx": pc["gidx"],
        })
    return in_maps


def _run(nc, in_maps, trace=False):
    if trace:
        try:
            _install_ntff_hook()
        except Exception as e:  # degrade to untraced run
            print(f"ntff hook install failed: {e}", file=sys.stderr)
    from concourse.bass_utils import run_bass_kernel_spmd

    return run_bass_kernel_spmd(
        nc, in_maps, core_ids=list(range(N_CORES)), trace=trace,
    )


def kernel(x, edge_index, edge_weight, W1, b1, W2, b2, _want_trace=False):
    x = np.ascontiguousarray(np.asarray(x, np.float32))
    W1 = np.asarray(W1, np.float32)
    b1 = np.asarray(b1, np.float32)
    W2 = np.asarray(W2, np.float32)
    b2 = np.asarray(b2, np.float32)

    N = x.shape[0]
    per_core, layout = _preprocess(x, edge_index, edge_weight)
    nc = _build_program(layout)

    in_maps = _assemble_inputs(x, W1, b1, W2, b2, per_core, layout)
    res = _run(nc, in_maps, trace=_want_trace)

    out = np.empty((N, D_IN), np.float32)
    for c in range(N_CORES):
        cols = res.results[c]["out"]            # [128, tiles_tot*128] f-major
        perm = per_core[c]["perm"]
        valid = perm >= 0
        out[perm[valid]] = cols[:, valid].T

    kernel.last_results = res
    return out
